# revision 27
# baseline (speedup 1.0000x reference)
"""Trainium2 Bass kernel for nn_MultiHeadAttention_66322884984909.

Math (faithful to reference):
  Q = X @ W_Q.T reshaped (B, H, L, hd) via DIRECT reshape -> head h owns rows
  128h:128(h+1) of the projected (L, D) matrix, reinterpreted as (L=2048, hd=64).
  Heads are sequence-parallel: 32 (batch, head) pairs, 8 cores x 4 pairs.

Design (v3 — single fused PE stream):
  - Q/K computed PRE-TRANSPOSED: Qf^T = W_Q.T' @ X_s^T with output partitions
    = W-output-columns; psum quarters land in qht2/kht2[64i+j, s=16r+2c+par]
    via strided DVE copies (no DRAM shuffle bounce, no PE transposes).
  - Causal masking without inner-loop affine_select: diagonal S tiles exp and
    PV read only columns q >= 128m of each key block; the 128-wide boundary
    strip is zeroed by one bf16 DVE multiply with a constant 0/1 triangle.
  - ONE software-pipelined PE stream for everything: DMA order X, WV, WQ, WK
    lets V(pairs 0,1) projections chase the WV chunks, then Q/K group-0
    projections chase WQ/WK; attention begins ~22us in.  Group-1's Q/K
    projections, pairs 2/3's V path, and the output projections are emitted
    as BACKGROUND CLOSURES injected between attention units, so the
    ACT-bound attention stream and the remaining PE work overlap instead of
    running as separate phases (v2 ran all projections before attention and
    idled the PE ~37us total).
  - PSUM budget (8 banks): stA/stB [128,1024] (2 banks each) for the q0/k0
    projection chunks and the attention S tiles; tag "pv" (2 bufs) for the
    PV accumulation chains; tags "tr"/"prj" (1 buf each) for transient
    V-proj / output-proj psums and injected [128,512] projection sub-chunks.
    Attention units are PAIR-SERIAL ((g, i, a, gg)) so only one PV chain is
    live at a time and each pair's output projection can inject mid-stream.
  - Softmax row sums ride as a 65th ones-column of V; normalization
    (reciprocal + partition_broadcast + strided muls) scatters O into the
    o2 layout read by the output projection.
  - bf16 inputs and weights throughout; S and psum accumulations stay fp32
    (fp32r matmuls with free dim >= 256 run at full rate).  No
    max-subtraction in softmax: logits ~ N(0, 64), exp stays finite in fp32.
"""

import numpy as np

import concourse.bass as bass
from concourse import bacc
import concourse.mybir as mybir
import concourse.tile as tile
from concourse.bass_utils import run_bass_kernel_spmd

F32 = mybir.dt.float32
F32R = mybir.dt.float32r
BF16 = mybir.dt.bfloat16
EXP = mybir.ActivationFunctionType.Exp

B, L, D = 2, 2048, 1024
H, HD = 16, 64
NCORES = 8
PPC = 4   # pairs per core
NG = 2    # groups of 2 pairs

INJ_GAP = 2   # attention units between background-closure injections


def build_nc(repeat=1):
    nc = bacc.Bacc(trn_type="TRN2", target_bir_lowering=False, debug=False)

    # xtb[p_, kc*512 + 128*pair + r] = X_pair[r, 128*kc + p_] (bf16)
    xtb = nc.declare_dram_parameter("xtb", [128, 8 * PPC * 128], BF16,
                                    isOutput=False)
    wq = nc.declare_dram_parameter("wq", [1024, 1024], BF16, isOutput=False)
    wk = nc.declare_dram_parameter("wk", [1024, 1024], BF16, isOutput=False)
    wv = nc.declare_dram_parameter("wv", [1024, 1024], BF16, isOutput=False)
    # wo[64*parity + j2, 1024*u + jo] = W_O.T[64*(2u+parity) + j2, jo]
    wo = nc.declare_dram_parameter("wo", [128, 8 * 1024], BF16, isOutput=False)
    bias = nc.declare_dram_parameter("bias", [1, 1024], F32, isOutput=False)
    out = nc.declare_dram_parameter("out", [PPC, 128, 1024], BF16,
                                    isOutput=True)
    vsh = nc.dram_tensor("vsh", [PPC, 128, 1040], BF16)

    with tile.TileContext(nc) as tc:
      for _rep in range(repeat):
        with (
            tc.tile_pool(name="consts", bufs=1) as consts,
            tc.tile_pool(name="headt", bufs=1) as headt,
            tc.tile_pool(name="mmps", bufs=1, space="PSUM") as mmps,
            tc.tile_pool(name="stps", bufs=1, space="PSUM") as stps,
        ):
            bias_sb = consts.tile([128, 1024], F32)
            # tri01[k, q'] = 1.0 if q' >= k else 0.0 (strip causal mask)
            tri01 = consts.tile([128, 128], BF16)
            nc.gpsimd.memset(tri01, 1.0)
            # dummy exp to preload the activation table during the
            # projection phase (the lazy load costs 1.3us otherwise)
            warm = consts.tile([128, 1], F32, tag="warm")
            nc.gpsimd.memset(warm, 0.0)
            nc.scalar.activation(warm, warm, EXP)
            # dummy matmuls ramp the PE p-state (full clock needs ~3us of
            # continuous execution) while the first DMAs stream in
            wmm = consts.tile([128, 256], F32, tag="wmm")
            nc.gpsimd.memset(wmm, 0.0)
            for _w in range(4):
                pw = mmps.tile([128, 256], F32, tag="tr", bufs=1,
                               name=f"warmmm{_w}")
                nc.tensor.matmul(pw, lhsT=wmm[:, 0:128], rhs=wmm,
                                 start=True, stop=True)
            nc.gpsimd.affine_select(
                out=tri01, in_=tri01,
                compare_op=mybir.AluOpType.is_ge,
                fill=0.0,
                base=0,
                pattern=[[1, 128]],
                channel_multiplier=-1,
            )

            # qht2/kht2[g]: [64*i + j, s] for pair 2g+i  (j = head dim)
            qht2 = [headt.tile([128, 2048], F32R, tag=f"qht{g}", name=f"qht{g}")
                    for g in range(NG)]
            kht2 = [headt.tile([128, 2048], F32R, tag=f"kht{g}", name=f"kht{g}")
                    for g in range(NG)]
            # vh[p]: [s-in-block, 65*bb + j] with ones column at j=64
            vh = [headt.tile([128, 16 * 65], BF16, tag=f"vh{p}", name=f"vh{p}")
                  for p in range(PPC)]

            def qk_chunks_full(w_sb, xq_sb, dsts, g, wname):
                """Transposed projection as 2 half-closures per (W, g), each
                one [128, 1024] psum chunk (2 banks) on a stA/stB tag.

                Quarter c = 4*half + c4 holds [(parity, j), (i, r)] with
                W-output column 128c + 64*parity + j; copies land in
                dsts[g][64i + j, s = 16r + 2c + parity].  psum start/stop
                act on whole 2KB banks: only the first quarter of each bank
                starts / last quarter stops its accumulation group.
                """
                def mk(half):
                    def emit(tag="stA"):
                        pg = stps.tile(
                            [128, 1024], F32, tag=tag,
                            name=f"prj_{wname}_{g}_{half}")
                        for kc in range(8):
                            for c4 in range(4):
                                c = 4 * half + c4
                                nc.tensor.matmul(
                                    pg[:, 256 * c4:256 * c4 + 256],
                                    lhsT=w_sb[:, kc, c * 128:(c + 1) * 128],
                                    rhs=xq_sb[:, kc, 2 * g:2 * g + 2, :],
                                    start=(kc == 0 and c4 % 2 == 0),
                                    stop=(kc == 7 and c4 % 2 == 1),
                                )
                        v = dsts[g].rearrange(
                            "(i j) (r c two) -> i j two c r", i=2, c=8, two=2)
                        pv4 = pg.rearrange(
                            "p (c4 two r) -> p c4 two r", c4=4, two=2)
                        for parity in range(2):
                            for i in range(2):
                                nc.vector.tensor_copy(
                                    v[i, :, parity, 4 * half:4 * half + 4, :],
                                    pv4[64 * parity:64 * parity + 64, :, i, :])
                    return emit
                return [mk(h) for h in range(2)]

            def qk_chunks_half(w_sb, xq_sb, dsts, g, wname):
                """Same projection as 4 quarter-closures of [128, 512] psum
                (1 bank, tag "prj") — injectable into the attention stream
                without touching the stA/stB rotation."""
                def mk(sub):
                    def emit(tag=None):
                        pg = mmps.tile(
                            [128, 512], F32, tag="prj", bufs=1,
                            name=f"prj_{wname}_{g}_{sub}")
                        for kc in range(8):
                            for c4 in range(2):
                                c = 2 * sub + c4
                                nc.tensor.matmul(
                                    pg[:, 256 * c4:256 * c4 + 256],
                                    lhsT=w_sb[:, kc, c * 128:(c + 1) * 128],
                                    rhs=xq_sb[:, kc, 2 * g:2 * g + 2, :],
                                    start=(kc == 0 and c4 == 0),
                                    stop=(kc == 7 and c4 == 1),
                                )
                        v = dsts[g].rearrange(
                            "(i j) (r c two) -> i j two c r", i=2, c=8, two=2)
                        pv4 = pg.rearrange(
                            "p (c4 two r) -> p c4 two r", c4=2, two=2)
                        for parity in range(2):
                            for i in range(2):
                                nc.vector.tensor_copy(
                                    v[i, :, parity, 2 * sub:2 * sub + 2, :],
                                    pv4[64 * parity:64 * parity + 64, :, i, :])
                    return emit
                return [mk(s) for s in range(4)]

            nats = {}

            def v_finish(p, pwork, weng=None, reng=None):
                """Copy-to-nat tail + DRAM shuffle bounce for pair p.  Pairs
                0/1 ride the Activation HWDGE queue (round-robins fairly
                against the SP weight stream in the cross-queue arbiter;
                Pool/SWDGE transfers lose it); pairs 2/3 use gpsimd SWDGE +
                SP read mid-attention when the FIFO is empty."""
                nat = nats[p]
                shr = vsh[p].rearrange(
                    "(il pp2) (t j) -> t il pp2 j",
                    il=8, t=16)[:, :, :, 0:64]
                (weng or nc.gpsimd).dma_start(out=shr, in_=nat[:])
                (reng or nc.sync).dma_start(out=vh[p][:], in_=vsh[p])
                nc.gpsimd.memset(
                    vh[p].rearrange(
                        "q (b c) -> q b c", c=65)[:, :, 64],
                    1.0)  # ones column at 65b+64

            def v_chunks_chase(wv_sb, xb_sb, pwork, pairs, tags):
                """V projection for `pairs` with kc as the OUTER loop across
                all 4 (pair, jh) psum streams, so the matmuls chase the WV
                chunk DMAs at full density.  Emitted inline (pre-attention).
                """
                pjs = [(p, jh) for p in pairs for jh in range(2)]
                pss = {}
                for (p, jh), tg in zip(pjs, tags):
                    pss[(p, jh)] = mmps.tile(
                        [128, 512], F32, tag=tg,
                        bufs=2 if tg == "pv" else 1,
                        name=f"vps{p}_{jh}")
                for p in pairs:
                    nats[p] = pwork.tile([128, 1024], BF16, tag="natv",
                                         bufs=2, name=f"natv{p}")
                for kc in range(8):
                    for (p, jh) in pjs:
                        nc.tensor.matmul(
                            pss[(p, jh)],
                            lhsT=xb_sb[:, kc, p, :],
                            rhs=wv_sb[:, kc, jh * 512:(jh + 1) * 512],
                            start=(kc == 0), stop=(kc == 7),
                        )
                for p in pairs:
                    for jh in range(2):
                        nc.vector.tensor_copy(
                            nats[p][:, jh * 512:(jh + 1) * 512],
                            pss[(p, jh)])
                    v_finish(p, pwork, weng=nc.scalar, reng=nc.scalar)

            def v_chunks_seq(wv_sb, xb_sb, pwork, pairs):
                """V projection as injectable closures, one per (pair, jh),
                each a [128, 512] psum (tag "tr")."""
                def mk(p, jh):
                    def emit(tag=None):
                        if jh == 0:
                            nats[p] = pwork.tile([128, 1024], BF16,
                                                 tag="natv", bufs=2,
                                                 name=f"natv{p}")
                        ps = mmps.tile([128, 512], F32, tag="tr", bufs=1,
                                       name=f"vps{p}_{jh}")
                        for kc in range(8):
                            nc.tensor.matmul(
                                ps,
                                lhsT=xb_sb[:, kc, p, :],
                                rhs=wv_sb[:, kc, jh * 512:(jh + 1) * 512],
                                start=(kc == 0), stop=(kc == 7),
                            )
                        nc.vector.tensor_copy(
                            nats[p][:, jh * 512:(jh + 1) * 512], ps)
                        if jh == 1:
                            v_finish(p, pwork)
                    return emit
                return [mk(p, jh) for p in pairs for jh in range(2)]

            def emit_y_closures(g, i, o2, wo_sb, ypool):
                """Output projection for pair (g, i) as 2 injectable
                closures (one per 512-column half)."""
                ysb = ypool.tile([128, 1024], BF16, tag="ysb", bufs=2,
                                 name=f"ysb{g}_{i}")

                def mk(jh):
                    def emit(tag=None):
                        yps = mmps.tile([128, 512], F32, tag="tr", bufs=1,
                                        name=f"ypsum_{g}_{i}_{jh}")
                        for u in range(8):
                            nc.tensor.matmul(
                                yps,
                                lhsT=o2[:, u * 128:(u + 1) * 128],
                                rhs=wo_sb[:, u * 1024 + jh * 512:
                                          u * 1024 + (jh + 1) * 512],
                                start=(u == 0), stop=(u == 7),
                            )
                        nc.vector.tensor_add(
                            ysb[:, jh * 512:(jh + 1) * 512], yps,
                            bias_sb[:, jh * 512:(jh + 1) * 512])
                        nc.sync.dma_start(
                            out=out[2 * g + i][:, jh * 512:(jh + 1) * 512],
                            in_=ysb[:, jh * 512:(jh + 1) * 512])
                    return emit
                return [mk(0), mk(1)]

            def emit_attention_fused(ptp, rp, o2p, ypool, wo_sb, background):
                """All 4 pairs' attention as ONE software-pipelined stream
                (pair-serial units), with `background` closures (group-1
                projections, V pairs 2/3, output projections) injected
                between units.  PV matmuls for unit n are emitted AFTER the
                S matmuls of unit n+1; sts psum tiles double-buffer via the
                stA/stB tag rotation."""
                o2s = {(g, i): o2p.tile([128, 8 * 128], BF16, tag=f"o2_{i}",
                                        name=f"o2_{g}_{i}")
                       for g in range(NG) for i in range(2)}
                pvs_by = {}

                def emit_pv(g, i, a, gg, pt):
                    if gg == 0:
                        pvs_by[(g, i, a)] = mmps.tile(
                            [65, 512], F32, tag="pv", bufs=2,
                            name=f"pv{g}_{i}_{a}")
                    pvs = pvs_by[(g, i, a)]
                    diag = gg >= 2 * a
                    d = gg - 2 * a
                    for q2 in range(2):
                        bb = 2 * gg + q2
                        if not diag:
                            nc.tensor.matmul(
                                pvs,
                                lhsT=vh[2 * g + i][:, bb * 65:bb * 65 + 65],
                                rhs=pt[:, q2 * 512:(q2 + 1) * 512],
                                start=(bb == 0),
                                stop=(bb == 4 * a + 3),
                            )
                        else:
                            m = 2 * d + q2
                            lo = q2 * 512 + 128 * m
                            nc.tensor.matmul(
                                pvs[:, 128 * m:512],
                                lhsT=vh[2 * g + i][:, bb * 65:bb * 65 + 65],
                                rhs=pt[:, lo:(q2 + 1) * 512],
                                start=(bb == 0),
                                stop=(bb == 4 * a + 3),
                                skip_group_check=True,
                            )

                def emit_norm(g, i, a):
                    pvs = pvs_by.pop((g, i, a))
                    r1 = rp.tile([1, 512], F32, tag="r1", name="r1_t")
                    nc.vector.reciprocal(r1, pvs[64:65, :])
                    rb = rp.tile([64, 512], F32, tag="rb", name="rb_t")
                    nc.gpsimd.partition_broadcast(rb, r1)
                    # o2[64*par + j2, 128u + 32a + r'] =
                    #     pvs[j2, 16r' + 2u + par] * rb[...]
                    pv_v = pvs[0:64, :].rearrange(
                        "j (rr uu two) -> j two uu rr", two=2, uu=8)
                    rb_v = rb.rearrange(
                        "j (rr uu two) -> j two uu rr", two=2, uu=8)
                    o2_v = o2s[(g, i)].rearrange(
                        "q (u rr) -> q u rr", u=8)[:, :, 32 * a:32 * a + 32]
                    for par in range(2):
                        nc.vector.tensor_mul(
                            o2_v[64 * par:64 * par + 64],
                            pv_v[:, par], rb_v[:, par])

                pending = None
                rot = [0]
                last_inj = [-INJ_GAP]

                def next_tag():
                    rot[0] += 1
                    return "stA" if rot[0] % 2 == 0 else "stB"

                units = [(g, i, a, gg)
                         for g in range(NG) for i in range(2)
                         for a in range(4) for gg in range(2 * a + 2)]
                n_units = len(units)
                for ui, (g, i, a, gg) in enumerate(units):
                    if (background and background[0][0] <= ui
                            and ui - last_inj[0] >= INJ_GAP):
                        last_inj[0] = ui
                        background.pop(0)[1]()
                    diag = gg >= 2 * a
                    d = gg - 2 * a
                    sts = stps.tile([128, 1024], F32, tag=next_tag(),
                                    name=f"st{g}_{i}_{a}_{gg}")
                    for q2 in range(2):
                        bb = 2 * gg + q2
                        # diagonal blocks m=1,2 only need cols >= 128m (m=3
                        # would drop the free dim under 256 for no gain)
                        m = 2 * d + q2 if diag else 0
                        off = 128 * m if m in (1, 2) else 0
                        nc.tensor.matmul(
                            sts[:, q2 * 512 + off:(q2 + 1) * 512],
                            lhsT=kht2[g][64 * i:64 * i + 64,
                                         bb * 128:(bb + 1) * 128],
                            rhs=qht2[g][64 * i:64 * i + 64,
                                        a * 512 + off:(a + 1) * 512],
                            start=True, stop=True,
                        )
                    pt = ptp.tile([128, 1024], BF16, tag="pt",
                                  name=f"pt_{g}_{i}_{a}_{gg}")
                    if not diag:
                        nc.scalar.activation(pt, sts, EXP)
                    elif d == 0:
                        # one full-width exp (cols [512:640) are garbage the
                        # restricted PV never reads); strips masked after
                        nc.scalar.activation(pt, sts, EXP)
                        nc.vector.tensor_mul(
                            pt[:, 0:128], pt[:, 0:128], tri01)
                        nc.vector.tensor_mul(
                            pt[:, 640:768], pt[:, 640:768], tri01)
                    else:
                        # exp only the valid columns; zero the boundary
                        # strip's upper triangle with the 0/1 mask
                        for q2 in range(2):
                            m = 2 * d + q2
                            lo = q2 * 512 + 128 * m
                            hi = (q2 + 1) * 512
                            nc.scalar.activation(
                                pt[:, lo:hi], sts[:, lo:hi], EXP)
                            nc.vector.tensor_mul(
                                pt[:, lo:lo + 128],
                                pt[:, lo:lo + 128], tri01)
                    if pending is not None:
                        pg_, pi_, pa_, pgg_, ppt_ = pending
                        emit_pv(pg_, pi_, pa_, pgg_, ppt_)
                        if pgg_ == 2 * pa_ + 1:
                            emit_norm(pg_, pi_, pa_)
                            if pa_ == 3:
                                # pair (pg_, pi_) finished: queue its output
                                # projection as background work
                                ycl = emit_y_closures(
                                    pg_, pi_, o2s[(pg_, pi_)], wo_sb, ypool)
                                for c in ycl:
                                    background.append((ui + 1, c))
                    pending = (g, i, a, gg, pt)
                emit_pv(pending[0], pending[1], pending[2], pending[3],
                        pending[4])
                emit_norm(pending[0], pending[1], pending[2])
                ycl = emit_y_closures(pending[0], pending[1],
                                      o2s[(pending[0], pending[1])],
                                      wo_sb, ypool)
                for c in ycl:
                    background.append((n_units, c))
                for _, c in background:  # flush whatever remains
                    c()

            with (
                tc.tile_pool(name="xtp", bufs=1) as xtp,
                tc.tile_pool(name="wp", bufs=1) as wp,
                tc.tile_pool(name="pwork", bufs=1) as pwork,
                tc.tile_pool(name="p2", bufs=1) as p2,
                tc.tile_pool(name="ptp", bufs=6) as ptp,
                tc.tile_pool(name="rp", bufs=2) as rp,
                tc.tile_pool(name="o2p", bufs=2) as o2p,
                tc.tile_pool(name="yp", bufs=2) as ypool,
            ):
                xb_sb = xtp.tile([128, 8, PPC, 128], BF16, tag="xb",
                                 name="xbsb")
                xv = xtb.rearrange("p (kc pr r) -> p kc pr r", kc=8, pr=PPC)

                # DMA order (all bulk loads on the otherwise-idle SP SEQ):
                # X/WV interleaved at 2-chunk granularity (the V projection
                # consumes both per kc), WQ, WK; bias + WO chunks are
                # injected late as background closures.  2-chunk transfers
                # halve the serial HWDGE descriptor-generation load.
                w_sbs = {}
                for wkey in ("wv", "wq", "wk"):
                    w_sbs[wkey] = wp.tile([128, 8, 1024], BF16,
                                          tag=f"w_{wkey}", name=f"w_{wkey}")

                def w_chunk_dma(eng, wkey, wparam, kc0, nkc=2):
                    eng.dma_start(
                        out=w_sbs[wkey][:, kc0:kc0 + nkc, :],
                        in_=wparam.rearrange(
                            "(c p) j -> p c j", p=128)[:, kc0:kc0 + nkc, :])

                # X + WV interleaved, then WQ, WK — all on the SP queue in
                # need-order (the scheduler keeps ready DMAs in emission
                # order); the v01 bounce rides the Activation queue.
                for kc in range(0, 8, 2):
                    nc.sync.dma_start(out=xb_sb[:, kc:kc + 2],
                                      in_=xv[:, kc:kc + 2])
                    w_chunk_dma(nc.sync, "wv", wv, kc)
                for kc in range(0, 8, 2):
                    w_chunk_dma(nc.sync, "wq", wq, kc)
                for kc in range(0, 8, 2):
                    w_chunk_dma(nc.sync, "wk", wk, kc)
                bias_1r = consts.tile([1, 1024], F32, tag="bias1r")
                wo_sb = p2.tile([128, 8 * 1024], BF16, tag="wo")

                # V pairs 0,1: kc-outer chase across 4 psum streams; the
                # bounce (SP write + SP read) is emitted inline and
                # completes while the group-0 Q/K projections run
                v_chunks_chase(w_sbs["wv"], xb_sb, pwork, [0, 1],
                               ["pv", "pv", "tr", "prj"])
                # Q/K group 0: full [128,1024] chunks on the stA/stB tags
                q0 = qk_chunks_full(w_sbs["wq"], xb_sb, qht2, 0, "q")
                k0 = qk_chunks_full(w_sbs["wk"], xb_sb, kht2, 0, "k")
                q0[0]("stA")
                q0[1]("stB")
                k0[0]("stA")
                k0[1]("stB")

                def wo_load(lo, hi):
                    def emit(tag=None):
                        for u in range(lo, hi):
                            nc.sync.dma_start(
                                out=wo_sb[:, u * 1024:(u + 1) * 1024],
                                in_=wo[:, u * 1024:(u + 1) * 1024])
                    return emit

                def bias_load(tag=None):
                    nc.sync.dma_start(out=bias_1r, in_=bias[:])
                    nc.gpsimd.partition_broadcast(bias_sb, bias_1r)

                # background: group-1 projections, late weight loads, V
                # pairs 2/3; per-pair output projections are appended by
                # the attention loop as pairs finish
                background = []
                for c in qk_chunks_half(w_sbs["wq"], xb_sb, qht2, 1, "q"):
                    background.append((0, c))
                background.append((0, wo_load(0, 4)))
                for c in qk_chunks_half(w_sbs["wk"], xb_sb, kht2, 1, "k"):
                    background.append((0, c))
                background.append((0, wo_load(4, 8)))
                background.append((0, bias_load))
                for c in v_chunks_seq(w_sbs["wv"], xb_sb, pwork, [2, 3]):
                    background.append((0, c))

                emit_attention_fused(ptp, rp, o2p, ypool, wo_sb, background)

    nc.finalize()
    return nc


def _host_prep(input_seq_embs, W_Q, W_K, W_V, W_O, b_O):
    X = np.asarray(input_seq_embs, dtype=np.float32)
    WQ = np.asarray(W_Q, dtype=np.float32)
    WK = np.asarray(W_K, dtype=np.float32)
    WV = np.asarray(W_V, dtype=np.float32)
    WO = np.asarray(W_O, dtype=np.float32)
    bO = np.asarray(b_O, dtype=np.float32)

    import ml_dtypes
    bf16 = ml_dtypes.bfloat16

    wq_arr = np.ascontiguousarray(WQ.T).astype(bf16)
    wk_arr = np.ascontiguousarray(WK.T).astype(bf16)
    wv_arr = np.ascontiguousarray(WV.T).astype(bf16)
    # wo[64*parity + j2, 1024*u + jo] = W_O.T[64*(2u+parity) + j2, jo]
    wo_arr = np.ascontiguousarray(
        WO.T.reshape(8, 2, 64, 1024).transpose(1, 2, 0, 3).reshape(
            128, 8192)).astype(bf16)
    bias_arr = np.ascontiguousarray(bO.reshape(1, 1024).astype(np.float32))

    in_maps = []
    for c in range(NCORES):
        # xt[p_, kc, pair, r] = X_pair[r, 128*kc + p_]
        xts = np.empty((128, 8, PPC, 128), dtype=np.float32)
        for p in range(PPC):
            g = PPC * c + p
            bb, hh = g // H, g % H
            Xs = X[bb, 128 * hh:128 * (hh + 1), :]      # (128 r, 1024 cin)
            xts[:, :, p, :] = Xs.T.reshape(8, 128, 128).transpose(1, 0, 2)
        xt_arr = np.ascontiguousarray(xts.reshape(128, 8 * PPC * 128))
        in_maps.append({
            "xtb": xt_arr.astype(bf16),
            "wq": wq_arr, "wk": wk_arr, "wv": wv_arr, "wo": wo_arr,
            "bias": bias_arr,
        })
    return in_maps


_CACHED_NC = None


def get_nc():
    global _CACHED_NC
    if _CACHED_NC is None:
        _CACHED_NC = build_nc()
    return _CACHED_NC


def kernel(**inputs) -> np.ndarray:
    nc = get_nc()
    in_maps = _host_prep(**inputs)
    res = run_bass_kernel_spmd(nc, in_maps, list(range(NCORES)))
    out = np.empty((B, L, D), dtype=np.float32)
    for c in range(NCORES):
        y = np.asarray(res.results[c]["out"],
                       dtype=np.float32)  # (4, 128, 1024)
        for p in range(PPC):
            g = PPC * c + p
            bb, hh = g // H, g % H
            out[bb, 128 * hh:128 * (hh + 1), :] = y[p]
    return out


# revision 28
# speedup vs baseline: 1.0461x; 1.0461x over previous
"""Trainium2 Bass kernel for nn_MultiHeadAttention_66322884984909.

Math (faithful to reference):
  Q = X @ W_Q.T reshaped (B, H, L, hd) via DIRECT reshape -> head h owns rows
  128h:128(h+1) of the projected (L, D) matrix, reinterpreted as (L=2048, hd=64).
  Heads are sequence-parallel: 32 (batch, head) pairs, 8 cores x 4 pairs.

Design (v3 — single fused PE stream):
  - Q/K computed PRE-TRANSPOSED: Qf^T = W_Q.T' @ X_s^T with output partitions
    = W-output-columns; psum quarters land in qht2/kht2[64i+j, s=16r+2c+par]
    via strided DVE copies (no DRAM shuffle bounce, no PE transposes).
  - Causal masking without inner-loop affine_select: diagonal S tiles exp and
    PV read only columns q >= 128m of each key block; the 128-wide boundary
    strip is zeroed by one bf16 DVE multiply with a constant 0/1 triangle.
  - ONE software-pipelined PE stream for everything: DMA order X, WV, WQ, WK
    lets V(pairs 0,1) projections chase the WV chunks, then Q/K group-0
    projections chase WQ/WK; attention begins ~22us in.  Group-1's Q/K
    projections, pairs 2/3's V path, and the output projections are emitted
    as BACKGROUND CLOSURES injected between attention units, so the
    ACT-bound attention stream and the remaining PE work overlap instead of
    running as separate phases (v2 ran all projections before attention and
    idled the PE ~37us total).
  - PSUM budget (8 banks): stA/stB [128,1024] (2 banks each) for the q0/k0
    projection chunks and the attention S tiles; tag "pv" (2 bufs) for the
    PV accumulation chains; tags "tr"/"prj" (1 buf each) for transient
    V-proj / output-proj psums and injected [128,512] projection sub-chunks.
    Attention units are PAIR-SERIAL ((g, i, a, gg)) so only one PV chain is
    live at a time and each pair's output projection can inject mid-stream.
  - Softmax row sums ride as a 65th ones-column of V; normalization
    (reciprocal + partition_broadcast + strided muls) scatters O into the
    o2 layout read by the output projection.
  - bf16 inputs and weights throughout; S and psum accumulations stay fp32
    (fp32r matmuls with free dim >= 256 run at full rate).  No
    max-subtraction in softmax: logits ~ N(0, 64), exp stays finite in fp32.
"""

import numpy as np

import concourse.bass as bass
from concourse import bacc
import concourse.mybir as mybir
import concourse.tile as tile
from concourse.bass_utils import run_bass_kernel_spmd

F32 = mybir.dt.float32
F32R = mybir.dt.float32r
BF16 = mybir.dt.bfloat16
EXP = mybir.ActivationFunctionType.Exp

B, L, D = 2, 2048, 1024
H, HD = 16, 64
NCORES = 8
PPC = 4   # pairs per core
NG = 2    # groups of 2 pairs

INJ_GAP = 2   # attention units between background-closure injections


def build_nc(repeat=1):
    nc = bacc.Bacc(trn_type="TRN2", target_bir_lowering=False, debug=False)

    # xtb[p_, kc*512 + 128*pair + r] = X_pair[r, 128*kc + p_] (bf16)
    xtb = nc.declare_dram_parameter("xtb", [128, 8 * PPC * 128], BF16,
                                    isOutput=False)
    wq = nc.declare_dram_parameter("wq", [1024, 1024], BF16, isOutput=False)
    wk = nc.declare_dram_parameter("wk", [1024, 1024], BF16, isOutput=False)
    wv = nc.declare_dram_parameter("wv", [1024, 1024], BF16, isOutput=False)
    # wo[64*parity + j2, 1024*u + jo] = W_O.T[64*(2u+parity) + j2, jo]
    wo = nc.declare_dram_parameter("wo", [128, 8 * 1024], BF16, isOutput=False)
    bias = nc.declare_dram_parameter("bias", [1, 1024], F32, isOutput=False)
    out = nc.declare_dram_parameter("out", [PPC, 128, 1024], BF16,
                                    isOutput=True)
    vsh = nc.dram_tensor("vsh", [PPC, 128, 1040], BF16)

    with tile.TileContext(nc) as tc:
      for _rep in range(repeat):
        with (
            tc.tile_pool(name="consts", bufs=1) as consts,
            tc.tile_pool(name="headt", bufs=1) as headt,
            tc.tile_pool(name="mmps", bufs=1, space="PSUM") as mmps,
            tc.tile_pool(name="stps", bufs=1, space="PSUM") as stps,
        ):
            bias_sb = consts.tile([128, 1024], F32)
            # tri01[k, q'] = 1.0 if q' >= k else 0.0 (strip causal mask)
            tri01 = consts.tile([128, 128], BF16)
            nc.gpsimd.memset(tri01, 1.0)
            # dummy exp to preload the activation table during the
            # projection phase (the lazy load costs 1.3us otherwise)
            warm = consts.tile([128, 1], F32, tag="warm")
            nc.gpsimd.memset(warm, 0.0)
            nc.scalar.activation(warm, warm, EXP)
            # dummy matmuls ramp the PE p-state (full clock needs ~3us of
            # continuous execution) while the first DMAs stream in
            wmm = consts.tile([128, 256], F32, tag="wmm")
            nc.gpsimd.memset(wmm, 0.0)
            for _w in range(4):
                pw = mmps.tile([128, 256], F32, tag="tr", bufs=1,
                               name=f"warmmm{_w}")
                nc.tensor.matmul(pw, lhsT=wmm[:, 0:128], rhs=wmm,
                                 start=True, stop=True)
            nc.gpsimd.affine_select(
                out=tri01, in_=tri01,
                compare_op=mybir.AluOpType.is_ge,
                fill=0.0,
                base=0,
                pattern=[[1, 128]],
                channel_multiplier=-1,
            )

            # qht2/kht2[g]: [64*i + j, s] for pair 2g+i  (j = head dim)
            qht2 = [headt.tile([128, 2048], F32R, tag=f"qht{g}", name=f"qht{g}")
                    for g in range(NG)]
            kht2 = [headt.tile([128, 2048], F32R, tag=f"kht{g}", name=f"kht{g}")
                    for g in range(NG)]
            # vh[p]: [s-in-block, 65*bb + j] with ones column at j=64
            vh = [headt.tile([128, 16 * 65], BF16, tag=f"vh{p}", name=f"vh{p}")
                  for p in range(PPC)]

            def qk_chunks_full(w_sb, xq_sb, dsts, g, wname):
                """Transposed projection as 2 half-closures per (W, g), each
                one [128, 1024] psum chunk (2 banks) on a stA/stB tag.

                Quarter c = 4*half + c4 holds [(parity, j), (i, r)] with
                W-output column 128c + 64*parity + j; copies land in
                dsts[g][64i + j, s = 16r + 2c + parity].  psum start/stop
                act on whole 2KB banks: only the first quarter of each bank
                starts / last quarter stops its accumulation group.
                """
                def mk(half):
                    def emit(tag="stA"):
                        pg = stps.tile(
                            [128, 1024], F32, tag=tag,
                            name=f"prj_{wname}_{g}_{half}")
                        for kc in range(8):
                            for c4 in range(4):
                                c = 4 * half + c4
                                nc.tensor.matmul(
                                    pg[:, 256 * c4:256 * c4 + 256],
                                    lhsT=w_sb[:, kc, c * 128:(c + 1) * 128],
                                    rhs=xq_sb[:, kc, 2 * g:2 * g + 2, :],
                                    start=(kc == 0 and c4 % 2 == 0),
                                    stop=(kc == 7 and c4 % 2 == 1),
                                )
                        v = dsts[g].rearrange(
                            "(i j) (r c two) -> i j two c r", i=2, c=8, two=2)
                        pv4 = pg.rearrange(
                            "p (c4 two r) -> p c4 two r", c4=4, two=2)
                        for parity in range(2):
                            for i in range(2):
                                nc.vector.tensor_copy(
                                    v[i, :, parity, 4 * half:4 * half + 4, :],
                                    pv4[64 * parity:64 * parity + 64, :, i, :])
                    return emit
                return [mk(h) for h in range(2)]

            def qk_chunks_half(w_sb, xq_sb, dsts, g, wname):
                """Same projection as 4 quarter-closures of [128, 512] psum
                (1 bank, tag "prj") — injectable into the attention stream
                without touching the stA/stB rotation."""
                def mk(sub):
                    def emit(tag=None):
                        pg = mmps.tile(
                            [128, 512], F32, tag="prj", bufs=1,
                            name=f"prj_{wname}_{g}_{sub}")
                        for kc in range(8):
                            for c4 in range(2):
                                c = 2 * sub + c4
                                nc.tensor.matmul(
                                    pg[:, 256 * c4:256 * c4 + 256],
                                    lhsT=w_sb[:, kc, c * 128:(c + 1) * 128],
                                    rhs=xq_sb[:, kc, 2 * g:2 * g + 2, :],
                                    start=(kc == 0 and c4 == 0),
                                    stop=(kc == 7 and c4 == 1),
                                )
                        v = dsts[g].rearrange(
                            "(i j) (r c two) -> i j two c r", i=2, c=8, two=2)
                        pv4 = pg.rearrange(
                            "p (c4 two r) -> p c4 two r", c4=2, two=2)
                        for parity in range(2):
                            for i in range(2):
                                nc.vector.tensor_copy(
                                    v[i, :, parity, 2 * sub:2 * sub + 2, :],
                                    pv4[64 * parity:64 * parity + 64, :, i, :])
                    return emit
                return [mk(s) for s in range(4)]

            nats = {}

            def v_finish(p, pwork, weng=None, reng=None):
                """Copy-to-nat tail + DRAM shuffle bounce for pair p.  Pairs
                0/1 ride the Activation HWDGE queue (round-robins fairly
                against the SP weight stream in the cross-queue arbiter;
                Pool/SWDGE transfers lose it); pairs 2/3 use gpsimd SWDGE +
                SP read mid-attention when the FIFO is empty."""
                nat = nats[p]
                shr = vsh[p].rearrange(
                    "(il pp2) (t j) -> t il pp2 j",
                    il=8, t=16)[:, :, :, 0:64]
                (weng or nc.gpsimd).dma_start(out=shr, in_=nat[:])
                (reng or nc.sync).dma_start(out=vh[p][:], in_=vsh[p])
                nc.gpsimd.memset(
                    vh[p].rearrange(
                        "q (b c) -> q b c", c=65)[:, :, 64],
                    1.0)  # ones column at 65b+64

            def v_chunks_chase(wv_sb, xb_sb, pwork, pairs, tags):
                """V projection for `pairs` with kc as the OUTER loop across
                all 4 (pair, jh) psum streams, so the matmuls chase the WV
                chunk DMAs at full density.  Emitted inline (pre-attention).
                """
                pjs = [(p, jh) for p in pairs for jh in range(2)]
                pss = {}
                for (p, jh), tg in zip(pjs, tags):
                    pss[(p, jh)] = mmps.tile(
                        [128, 512], F32, tag=tg,
                        bufs=2 if tg == "pv" else 1,
                        name=f"vps{p}_{jh}")
                for p in pairs:
                    nats[p] = pwork.tile([128, 1024], BF16, tag="natv",
                                         bufs=2, name=f"natv{p}")
                for kc in range(8):
                    for (p, jh) in pjs:
                        nc.tensor.matmul(
                            pss[(p, jh)],
                            lhsT=xb_sb[:, kc, p, :],
                            rhs=wv_sb[:, kc, jh * 512:(jh + 1) * 512],
                            start=(kc == 0), stop=(kc == 7),
                        )
                for p in pairs:
                    for jh in range(2):
                        nc.vector.tensor_copy(
                            nats[p][:, jh * 512:(jh + 1) * 512],
                            pss[(p, jh)])
                    v_finish(p, pwork, weng=nc.scalar, reng=nc.scalar)

            def v_chunks_seq(wv_sb, xb_sb, pwork, pairs):
                """V projection as injectable closures, one per (pair, jh),
                each a [128, 512] psum (tag "tr")."""
                def mk(p, jh):
                    def emit(tag=None):
                        if jh == 0:
                            nats[p] = pwork.tile([128, 1024], BF16,
                                                 tag="natv", bufs=2,
                                                 name=f"natv{p}")
                        ps = mmps.tile([128, 512], F32, tag="tr", bufs=1,
                                       name=f"vps{p}_{jh}")
                        for kc in range(8):
                            nc.tensor.matmul(
                                ps,
                                lhsT=xb_sb[:, kc, p, :],
                                rhs=wv_sb[:, kc, jh * 512:(jh + 1) * 512],
                                start=(kc == 0), stop=(kc == 7),
                            )
                        nc.vector.tensor_copy(
                            nats[p][:, jh * 512:(jh + 1) * 512], ps)
                        if jh == 1:
                            v_finish(p, pwork)
                    return emit
                return [mk(p, jh) for p in pairs for jh in range(2)]

            def emit_y_closures(g, i, o2, wo_sb, ypool):
                """Output projection for pair (g, i) as 2 injectable
                closures (one per 512-column half)."""
                ysb = ypool.tile([128, 1024], BF16, tag="ysb", bufs=2,
                                 name=f"ysb{g}_{i}")

                def mk(jh):
                    def emit(tag=None):
                        yps = mmps.tile([128, 512], F32, tag="tr", bufs=1,
                                        name=f"ypsum_{g}_{i}_{jh}")
                        for u in range(8):
                            nc.tensor.matmul(
                                yps,
                                lhsT=o2[:, u * 128:(u + 1) * 128],
                                rhs=wo_sb[:, u * 1024 + jh * 512:
                                          u * 1024 + (jh + 1) * 512],
                                start=(u == 0), stop=(u == 7),
                            )
                        nc.vector.tensor_add(
                            ysb[:, jh * 512:(jh + 1) * 512], yps,
                            bias_sb[:, jh * 512:(jh + 1) * 512])
                        nc.sync.dma_start(
                            out=out[2 * g + i][:, jh * 512:(jh + 1) * 512],
                            in_=ysb[:, jh * 512:(jh + 1) * 512])
                    return emit
                return [mk(0), mk(1)]

            def emit_attention_fused(ptp, rp, o2p, ypool, wo_sb, background):
                """All 4 pairs' attention as ONE software-pipelined stream
                (pair-serial units), with `background` closures (group-1
                projections, V pairs 2/3, output projections) injected
                between units.  PV matmuls for unit n are emitted AFTER the
                S matmuls of unit n+1; sts psum tiles double-buffer via the
                stA/stB tag rotation."""
                o2s = {(g, i): o2p.tile([128, 8 * 128], BF16, tag=f"o2_{i}",
                                        name=f"o2_{g}_{i}")
                       for g in range(NG) for i in range(2)}
                pvs_by = {}

                def emit_pv(g, i, a, gg, pt):
                    if gg == 0:
                        pvs_by[(g, i, a)] = mmps.tile(
                            [65, 512], F32, tag="pv", bufs=2,
                            name=f"pv{g}_{i}_{a}")
                    pvs = pvs_by[(g, i, a)]
                    diag = gg >= 2 * a
                    d = gg - 2 * a
                    for q2 in range(2):
                        bb = 2 * gg + q2
                        if not diag:
                            nc.tensor.matmul(
                                pvs,
                                lhsT=vh[2 * g + i][:, bb * 65:bb * 65 + 65],
                                rhs=pt[:, q2 * 512:(q2 + 1) * 512],
                                start=(bb == 0),
                                stop=(bb == 4 * a + 3),
                            )
                        else:
                            m = 2 * d + q2
                            lo = q2 * 512 + 128 * m
                            nc.tensor.matmul(
                                pvs[:, 128 * m:512],
                                lhsT=vh[2 * g + i][:, bb * 65:bb * 65 + 65],
                                rhs=pt[:, lo:(q2 + 1) * 512],
                                start=(bb == 0),
                                stop=(bb == 4 * a + 3),
                                skip_group_check=True,
                            )

                def emit_norm(g, i, a):
                    pvs = pvs_by.pop((g, i, a))
                    r1 = rp.tile([1, 512], F32, tag="r1", name="r1_t")
                    nc.vector.reciprocal(r1, pvs[64:65, :])
                    rb = rp.tile([64, 512], F32, tag="rb", name="rb_t")
                    nc.gpsimd.partition_broadcast(rb, r1)
                    # o2[64*par + j2, 128u + 32a + r'] =
                    #     pvs[j2, 16r' + 2u + par] * rb[...]
                    pv_v = pvs[0:64, :].rearrange(
                        "j (rr uu two) -> j two uu rr", two=2, uu=8)
                    rb_v = rb.rearrange(
                        "j (rr uu two) -> j two uu rr", two=2, uu=8)
                    o2_v = o2s[(g, i)].rearrange(
                        "q (u rr) -> q u rr", u=8)[:, :, 32 * a:32 * a + 32]
                    for par in range(2):
                        nc.vector.tensor_mul(
                            o2_v[64 * par:64 * par + 64],
                            pv_v[:, par], rb_v[:, par])

                pending = None
                rot = [0]
                last_inj = [-INJ_GAP]

                def next_tag():
                    rot[0] += 1
                    return "stA" if rot[0] % 2 == 0 else "stB"

                units = [(g, i, a, gg)
                         for g in range(NG) for i in range(2)
                         for a in range(4) for gg in range(2 * a + 2)]
                n_units = len(units)
                for ui, (g, i, a, gg) in enumerate(units):
                    if (background and background[0][0] <= ui
                            and ui - last_inj[0] >= INJ_GAP):
                        last_inj[0] = ui
                        background.pop(0)[1]()
                    diag = gg >= 2 * a
                    d = gg - 2 * a
                    sts = stps.tile([128, 1024], F32, tag=next_tag(),
                                    name=f"st{g}_{i}_{a}_{gg}")
                    for q2 in range(2):
                        bb = 2 * gg + q2
                        # diagonal blocks m=1,2 only need cols >= 128m (m=3
                        # would drop the free dim under 256 for no gain)
                        m = 2 * d + q2 if diag else 0
                        off = 128 * m if m in (1, 2) else 0
                        nc.tensor.matmul(
                            sts[:, q2 * 512 + off:(q2 + 1) * 512],
                            lhsT=kht2[g][64 * i:64 * i + 64,
                                         bb * 128:(bb + 1) * 128],
                            rhs=qht2[g][64 * i:64 * i + 64,
                                        a * 512 + off:(a + 1) * 512],
                            start=True, stop=True,
                        )
                    pt = ptp.tile([128, 1024], BF16, tag="pt",
                                  name=f"pt_{g}_{i}_{a}_{gg}")
                    if not diag:
                        nc.scalar.activation(pt, sts, EXP)
                    elif d == 0:
                        # one full-width exp (cols [512:640) are garbage the
                        # restricted PV never reads); strips masked after
                        nc.scalar.activation(pt, sts, EXP)
                        nc.vector.tensor_mul(
                            pt[:, 0:128], pt[:, 0:128], tri01)
                        nc.vector.tensor_mul(
                            pt[:, 640:768], pt[:, 640:768], tri01)
                    else:
                        # exp only the valid columns; zero the boundary
                        # strip's upper triangle with the 0/1 mask
                        for q2 in range(2):
                            m = 2 * d + q2
                            lo = q2 * 512 + 128 * m
                            hi = (q2 + 1) * 512
                            nc.scalar.activation(
                                pt[:, lo:hi], sts[:, lo:hi], EXP)
                            nc.vector.tensor_mul(
                                pt[:, lo:lo + 128],
                                pt[:, lo:lo + 128], tri01)
                    if pending is not None:
                        pg_, pi_, pa_, pgg_, ppt_ = pending
                        emit_pv(pg_, pi_, pa_, pgg_, ppt_)
                        if pgg_ == 2 * pa_ + 1:
                            emit_norm(pg_, pi_, pa_)
                            if pa_ == 3:
                                # pair (pg_, pi_) finished: queue its output
                                # projection as background work
                                ycl = emit_y_closures(
                                    pg_, pi_, o2s[(pg_, pi_)], wo_sb, ypool)
                                for c in ycl:
                                    background.append((ui + 1, c))
                    pending = (g, i, a, gg, pt)
                emit_pv(pending[0], pending[1], pending[2], pending[3],
                        pending[4])
                emit_norm(pending[0], pending[1], pending[2])
                ycl = emit_y_closures(pending[0], pending[1],
                                      o2s[(pending[0], pending[1])],
                                      wo_sb, ypool)
                for c in ycl:
                    background.append((n_units, c))
                for _, c in background:  # flush whatever remains
                    c()

            with (
                tc.tile_pool(name="xtp", bufs=1) as xtp,
                tc.tile_pool(name="wp", bufs=1) as wp,
                tc.tile_pool(name="pwork", bufs=1) as pwork,
                tc.tile_pool(name="p2", bufs=1) as p2,
                tc.tile_pool(name="ptp", bufs=6) as ptp,
                tc.tile_pool(name="rp", bufs=2) as rp,
                tc.tile_pool(name="o2p", bufs=2) as o2p,
                tc.tile_pool(name="yp", bufs=2) as ypool,
            ):
                xb_sb = xtp.tile([128, 8, PPC, 128], BF16, tag="xb",
                                 name="xbsb")
                xv = xtb.rearrange("p (kc pr r) -> p kc pr r", kc=8, pr=PPC)

                # DMA order (all bulk loads on the otherwise-idle SP SEQ):
                # X/WV interleaved at 2-chunk granularity (the V projection
                # consumes both per kc), WQ, WK; bias + WO chunks are
                # injected late as background closures.  2-chunk transfers
                # halve the serial HWDGE descriptor-generation load.
                w_sbs = {}
                for wkey in ("wv", "wq", "wk"):
                    w_sbs[wkey] = wp.tile([128, 8, 1024], BF16,
                                          tag=f"w_{wkey}", name=f"w_{wkey}")

                def w_chunk_dma(eng, wkey, wparam, kc0, nkc=2):
                    eng.dma_start(
                        out=w_sbs[wkey][:, kc0:kc0 + nkc, :],
                        in_=wparam.rearrange(
                            "(c p) j -> p c j", p=128)[:, kc0:kc0 + nkc, :])

                # X + WV interleaved, then WQ, WK — all on the SP queue in
                # need-order (the scheduler keeps ready DMAs in emission
                # order); the v01 bounce rides the Activation queue.
                for kc in range(0, 8, 2):
                    nc.sync.dma_start(out=xb_sb[:, kc:kc + 2],
                                      in_=xv[:, kc:kc + 2])
                    w_chunk_dma(nc.sync, "wv", wv, kc)
                for kc in range(0, 8, 2):
                    w_chunk_dma(nc.sync, "wq", wq, kc)
                for kc in range(0, 8, 2):
                    w_chunk_dma(nc.sync, "wk", wk, kc)
                bias_1r = consts.tile([1, 1024], F32, tag="bias1r")
                wo_sb = p2.tile([128, 8 * 1024], BF16, tag="wo")

                # V pairs 0,1: kc-outer chase across 4 psum streams; the
                # bounce (SP write + SP read) is emitted inline and
                # completes while the group-0 Q/K projections run
                v_chunks_chase(w_sbs["wv"], xb_sb, pwork, [0, 1],
                               ["pv", "pv", "tr", "prj"])
                # Q/K group 0: full [128,1024] chunks on the stA/stB tags
                q0 = qk_chunks_full(w_sbs["wq"], xb_sb, qht2, 0, "q")
                k0 = qk_chunks_full(w_sbs["wk"], xb_sb, kht2, 0, "k")
                q0[0]("stA")
                q0[1]("stB")
                k0[0]("stA")
                k0[1]("stB")

                def wo_load(lo, hi):
                    def emit(tag=None):
                        for u in range(lo, hi):
                            nc.sync.dma_start(
                                out=wo_sb[:, u * 1024:(u + 1) * 1024],
                                in_=wo[:, u * 1024:(u + 1) * 1024])
                    return emit

                def bias_load(tag=None):
                    nc.sync.dma_start(out=bias_1r, in_=bias[:])
                    nc.gpsimd.partition_broadcast(bias_sb, bias_1r)

                # group-1 projections, V pairs 2/3, late weight loads —
                # emitted in dependency order before the attention stream;
                # the tile scheduler hoists/interleaves them into PE
                # bubbles itself.  Output projections are appended to
                # `background` by the attention loop as pairs finish.
                for c in qk_chunks_half(w_sbs["wq"], xb_sb, qht2, 1, "q"):
                    c()
                for c in qk_chunks_half(w_sbs["wk"], xb_sb, kht2, 1, "k"):
                    c()
                wo_load(0, 4)()
                wo_load(4, 8)()
                bias_load()
                for c in v_chunks_seq(w_sbs["wv"], xb_sb, pwork, [2, 3]):
                    c()

                background = []
                emit_attention_fused(ptp, rp, o2p, ypool, wo_sb, background)

    nc.finalize()
    return nc


def _host_prep(input_seq_embs, W_Q, W_K, W_V, W_O, b_O):
    X = np.asarray(input_seq_embs, dtype=np.float32)
    WQ = np.asarray(W_Q, dtype=np.float32)
    WK = np.asarray(W_K, dtype=np.float32)
    WV = np.asarray(W_V, dtype=np.float32)
    WO = np.asarray(W_O, dtype=np.float32)
    bO = np.asarray(b_O, dtype=np.float32)

    import ml_dtypes
    bf16 = ml_dtypes.bfloat16

    wq_arr = np.ascontiguousarray(WQ.T).astype(bf16)
    wk_arr = np.ascontiguousarray(WK.T).astype(bf16)
    wv_arr = np.ascontiguousarray(WV.T).astype(bf16)
    # wo[64*parity + j2, 1024*u + jo] = W_O.T[64*(2u+parity) + j2, jo]
    wo_arr = np.ascontiguousarray(
        WO.T.reshape(8, 2, 64, 1024).transpose(1, 2, 0, 3).reshape(
            128, 8192)).astype(bf16)
    bias_arr = np.ascontiguousarray(bO.reshape(1, 1024).astype(np.float32))

    in_maps = []
    for c in range(NCORES):
        # xt[p_, kc, pair, r] = X_pair[r, 128*kc + p_]
        xts = np.empty((128, 8, PPC, 128), dtype=np.float32)
        for p in range(PPC):
            g = PPC * c + p
            bb, hh = g // H, g % H
            Xs = X[bb, 128 * hh:128 * (hh + 1), :]      # (128 r, 1024 cin)
            xts[:, :, p, :] = Xs.T.reshape(8, 128, 128).transpose(1, 0, 2)
        xt_arr = np.ascontiguousarray(xts.reshape(128, 8 * PPC * 128))
        in_maps.append({
            "xtb": xt_arr.astype(bf16),
            "wq": wq_arr, "wk": wk_arr, "wv": wv_arr, "wo": wo_arr,
            "bias": bias_arr,
        })
    return in_maps


_CACHED_NC = None


def get_nc():
    global _CACHED_NC
    if _CACHED_NC is None:
        _CACHED_NC = build_nc()
    return _CACHED_NC


def kernel(**inputs) -> np.ndarray:
    nc = get_nc()
    in_maps = _host_prep(**inputs)
    res = run_bass_kernel_spmd(nc, in_maps, list(range(NCORES)))
    out = np.empty((B, L, D), dtype=np.float32)
    for c in range(NCORES):
        y = np.asarray(res.results[c]["out"],
                       dtype=np.float32)  # (4, 128, 1024)
        for p in range(PPC):
            g = PPC * c + p
            bb, hh = g // H, g % H
            out[bb, 128 * hh:128 * (hh + 1), :] = y[p]
    return out


# revision 33
# speedup vs baseline: 1.0684x; 1.0213x over previous
"""Trainium2 Bass kernel for nn_MultiHeadAttention_66322884984909.

Math (faithful to reference):
  Q = X @ W_Q.T reshaped (B, H, L, hd) via DIRECT reshape -> head h owns rows
  128h:128(h+1) of the projected (L, D) matrix, reinterpreted as (L=2048, hd=64).
  Heads are sequence-parallel: 32 (batch, head) pairs, 8 cores x 4 pairs.

Design:
  - Q/K computed PRE-TRANSPOSED: Qf^T = W_Q.T' @ X_s^T with output partitions
    = W-output-columns, batching a group's 2 pairs in the matmul free dim.
    psum quarters [(t-parity, j), (pair, r)] land in qht2/kht2[64i+j,
    s=16r+2c+parity] via strided DVE copies -- no DRAM shuffle bounce and no
    PE transposes for Q/K.  (psum start/stop act on whole 2KB banks: only the
    first/last quarter of a bank starts/stops its accumulation group.)
  - Causal masking without an inner-loop affine_select: on diagonal S tiles,
    exp and P@V read only columns q >= 128m of each key block; the 128-wide
    boundary strip is zeroed by one bf16 DVE multiply with a constant 0/1
    triangle.  The m=0 block is full width, so the psum accumulation group
    start stays uniform (bb == 0).
  - Attention is software-pipelined per unit (a-chunk, key-block-pair, pair):
    S(n+1) is emitted BEFORE PV(n) so the PE never idles on the exp latency;
    sts psum tiles double-buffer via a shared stA/stB tag rotation.
    Softmax row sums ride along as a 65th ones-column of V; normalization
    (reciprocal + partition_broadcast + strided muls) scatters O directly
    into the o2 layout [(s-parity, j2), (u, r)].
  - Output projection contracts 128-deep (8 accumulating matmuls per 512-col
    half), split into per-pair closures interleaved into group 1's
    attention stream.  Both groups' attention form ONE fused pipeline, so
    group 0's drain overlaps group 1's ramp-up.  W_V loads first and the
    descriptor-heavy V shuffle-scatter overlaps the W_Q/W_K streams;
    dummy matmuls/exp ramp the PE p-state and activation table during the
    initial DMA wait (kc is the outer loop so matmuls chase the W DMAs).
  - bf16 inputs and weights throughout (X, W_Q, W_K, W_V, W_O, the V DRAM
    bounce, exp output P, O): measured 1.05e-2 relative error against the
    2e-2 gate (softmax renormalization cancels most of the bf16 logit
    error); S and the psum accumulations stay fp32.  Halves the weight
    DMA lead-in that gates the attention start.
  - No max-subtraction in softmax: logits ~ N(0, 64); exp stays finite in
    fp32.  All fp32 matmuls run as fp32r with free dim >= 256 (full rate).

Cost-model (TimelineSim) total: 158.8 us vs 248.9 us for the v1 baseline.
"""

import numpy as np

import concourse.bass as bass
from concourse import bacc
import concourse.mybir as mybir
import concourse.tile as tile
from concourse.bass_utils import run_bass_kernel_spmd

F32 = mybir.dt.float32
F32R = mybir.dt.float32r
BF16 = mybir.dt.bfloat16
EXP = mybir.ActivationFunctionType.Exp

B, L, D = 2, 2048, 1024
H, HD = 16, 64
NCORES = 8
PPC = 4   # pairs per core
NG = 2    # groups of 2 pairs
NEG = -1.0e30


def build_nc(repeat=1):
    nc = bacc.Bacc(trn_type="TRN2", target_bir_lowering=False, debug=False)

    # xtb[p_, kc*512 + 128*pair + r] = X_pair[r, 128*kc + p_] (bf16)
    xtb = nc.declare_dram_parameter("xtb", [128, 8 * PPC * 128], BF16,
                                    isOutput=False)
    wq = nc.declare_dram_parameter("wq", [1024, 1024], BF16, isOutput=False)
    wk = nc.declare_dram_parameter("wk", [1024, 1024], BF16, isOutput=False)
    wv = nc.declare_dram_parameter("wv", [1024, 1024], BF16, isOutput=False)
    # wo[64*parity + j2, 1024*u + jo] = W_O.T[64*(2u+parity) + j2, jo]
    wo = nc.declare_dram_parameter("wo", [128, 8 * 1024], BF16, isOutput=False)
    bias = nc.declare_dram_parameter("bias", [128, 1024], F32, isOutput=False)
    out = nc.declare_dram_parameter("out", [PPC, 128, 1024], BF16,
                                    isOutput=True)
    vsh = nc.dram_tensor("vsh", [PPC, 128, 1040], BF16)

    with tile.TileContext(nc) as tc:
      for _rep in range(repeat):
        with (
            tc.tile_pool(name="consts", bufs=1) as consts,
            tc.tile_pool(name="headt", bufs=1) as headt,
            tc.tile_pool(name="mmps", bufs=4, space="PSUM") as mmps,
            tc.tile_pool(name="stps", bufs=1, space="PSUM") as stps,
        ):
            bias_sb = consts.tile([128, 1024], F32)
            # tri01[k, q'] = 1.0 if q' >= k else 0.0 (strip causal mask)
            tri01 = consts.tile([128, 128], BF16)
            nc.gpsimd.memset(tri01, 1.0)
            # dummy exp to preload the activation table during the
            # projection phase (the lazy load costs 1.3us otherwise)
            warm = consts.tile([128, 1], F32, tag="warm")
            nc.gpsimd.memset(warm, 0.0)
            nc.scalar.activation(warm, warm, EXP)
            # dummy matmuls ramp the PE p-state (full clock needs ~3us of
            # continuous execution) while the first weight DMAs stream in
            wmm = consts.tile([128, 256], F32, tag="wmm")
            nc.gpsimd.memset(wmm, 0.0)
            for _w in range(1):
                pw = mmps.tile([128, 256], F32, tag="mm", name=f"warmmm{_w}")
                nc.tensor.matmul(pw, lhsT=wmm[:, 0:128], rhs=wmm,
                                 start=True, stop=True)
            nc.gpsimd.affine_select(
                out=tri01, in_=tri01,
                compare_op=mybir.AluOpType.is_ge,
                fill=0.0,
                base=0,
                pattern=[[1, 128]],
                channel_multiplier=-1,
            )
            # identity matrix for the PE transposes in the PV epilogue
            ident = consts.tile([128, 128], BF16, tag="ident")
            nc.gpsimd.memset(ident, 1.0)
            nc.gpsimd.affine_select(
                out=ident, in_=ident,
                compare_op=mybir.AluOpType.is_equal,
                fill=0.0,
                base=0,
                pattern=[[1, 128]],
                channel_multiplier=-1,
            )

            # qht2/kht2[g]: [64*i + j, s] for pair 2g+i  (j = head dim)
            qht2 = [headt.tile([128, 2048], F32R, tag=f"qht{g}", name=f"qht{g}")
                    for g in range(NG)]
            kht2 = [headt.tile([128, 2048], F32R, tag=f"kht{g}", name=f"kht{g}")
                    for g in range(NG)]
            # vh[p]: [s-in-block, 65*bb + j] with ones column at j=64
            vh = [headt.tile([128, 16 * 65], BF16, tag=f"vh{p}", name=f"vh{p}")
                  for p in range(PPC)]

            def qk_chunks(w_sb, xq_sb, dsts, g, wname):
                """Transposed projection, as 4 chunk-closures per (W, g).

                Chunk (half, bank): one psum tile [128, 512] = 2 c-quarters
                (c = 4*half + 2*bank + cq).  Quarter c holds [(parity, j),
                (i, r)] with W-output column 128c + 64*parity + j; copies land
                in dsts[g][64i + j, s = 16r + 2c + parity].  psum start/stop
                act on whole 2KB banks, so only the first quarter starts and
                only the last stops the accumulation group.
                """
                def mk(half):
                    def emit(tag="stA"):
                        pg = stps.tile(
                            [128, 1024], F32, tag=tag,
                            name=f"prj_{wname}_{g}_{half}")
                        for kc in range(8):
                            for c4 in range(4):
                                c = 4 * half + c4
                                # psum start/stop act on whole 2KB banks: a
                                # bank spans two 256-col quarters, so only the
                                # first quarter starts / last quarter stops
                                nc.tensor.matmul(
                                    pg[:, 256 * c4:256 * c4 + 256],
                                    lhsT=w_sb[:, kc, c * 128:(c + 1) * 128],
                                    rhs=xq_sb[:, kc, 2 * g:2 * g + 2, :],
                                    start=(kc == 0 and c4 % 2 == 0),
                                    stop=(kc == 7 and c4 % 2 == 1),
                                )
                        v = dsts[g].rearrange(
                            "(i j) (r c two) -> i j two c r", i=2, c=8, two=2)
                        pv4 = pg.rearrange(
                            "p (c4 two r) -> p c4 two r", c4=4, two=2)
                        for parity in range(2):
                            for i in range(2):
                                nc.vector.tensor_copy(
                                    v[i, :, parity, 4 * half:4 * half + 4, :],
                                    pv4[64 * parity:64 * parity + 64, :, i, :])
                    return emit
                return [mk(h) for h in range(2)]

            def v_chunks(wv_sb, xb_sb, pwork, pairs):
                """V projection + DRAM shuffle bounce, one closure per
                (pair, jh-half)."""
                nats = {}

                def mk(p, jh):
                    def emit(tag=None):
                        if jh == 0:
                            nats[p] = pwork.tile([128, 1024], BF16, tag="natv",
                                                 bufs=2, name=f"natv{p}")
                        nat = nats[p]
                        ps = mmps.tile([128, 512], F32, tag="mm",
                                       name="projps")
                        for kc in range(8):
                            nc.tensor.matmul(
                                ps,
                                lhsT=xb_sb[:, kc, p, :],
                                rhs=wv_sb[:, kc, jh * 512:(jh + 1) * 512],
                                start=(kc == 0), stop=(kc == 7),
                            )
                        nc.vector.tensor_copy(
                            nat[:, jh * 512:(jh + 1) * 512], ps)
                        if jh == 1:
                            shr = vsh[p].rearrange(
                                "(il pp2) (t j) -> t il pp2 j",
                                il=8, t=16)[:, :, :, 0:64]
                            nc.gpsimd.dma_start(out=shr, in_=nat[:])
                            nc.scalar.dma_start(out=vh[p][:], in_=vsh[p])
                            nc.gpsimd.memset(
                                vh[p].rearrange(
                                    "q (b c) -> q b c", c=65)[:, :, 64],
                                1.0)  # ones column at 65b+64
                    return emit
                return [mk(p, jh) for p in pairs for jh in range(2)]

            def emit_attention_fused(ptp, rp, o2p, ypool, wo_sb):
                """Both groups' attention as ONE software-pipelined stream:
                the PV matmuls for unit n are emitted AFTER the S matmuls of
                unit n+1 (psum stA/stB rotate at depth 2), and the pipeline
                crosses the group boundary so group 0's drain overlaps group
                1's ramp-up.  Group 0's output projection is injected a few
                units after its last normalization."""
                o2s = {g: [o2p.tile([128, 8 * 128], BF16, tag=f"o2_{i}",
                                    name=f"o2_{g}_{i}") for i in range(2)]
                       for g in range(NG)}
                pvs_by = {}

                def emit_pv(g, a, gg, i, pt):
                    """PV with output partitions = queries: per (key block
                    bb, 128-query sub-block mm) one [128, 65] matmul (65
                    rows instead of 512 on the PE).  The four q-sub-block
                    chains of the a-chunk share one psum bank; only the
                    very first/last matmul starts/stops the bank's group.
                    Below-diagonal blocks (4a + mm < bb) are skipped, which
                    also skips the uncomputed pt regions."""
                    ch = pvs_by[(g, a)][i]
                    for q2 in range(2):
                        bb = 2 * gg + q2
                        for mm in range(4):
                            if 4 * a + mm < bb:
                                continue
                            lo = q2 * 512 + 128 * mm
                            nc.tensor.matmul(
                                ch[:, mm, :],
                                lhsT=pt[:, lo:lo + 128],
                                rhs=vh[2 * g + i][:, bb * 65:bb * 65 + 65],
                                start=(bb == 0 and mm == 0),
                                stop=(bb == 4 * a + 3 and mm == 3),
                            )

                def emit_norm_one(g, a, i):
                    """Normalize in [q, j] orientation (per-partition scalar
                    multiply by 1/Z), transpose the four 128-query blocks
                    back to [j, q] with the PE, and scatter into o2."""
                    ch = pvs_by[(g, a)][i]
                    recip4 = rp.tile([128, 4], F32, tag="r4", name="r4_t")
                    nc.vector.reciprocal(recip4, ch[:, :, 64])
                    stg = rp.tile([128, 4, 65], BF16, tag="stg",
                                  name="stg_t")
                    for mm in range(4):
                        nc.vector.tensor_scalar_mul(
                            stg[:, mm, :], ch[:, mm, :],
                            recip4[:, mm:mm + 1])
                    trp = mmps.tile([65, 512], BF16, tag="mm",
                                    name=f"trp{g}_{a}_{i}")
                    for mm in range(4):
                        nc.tensor.matmul(
                            trp[:, 128 * mm:128 * mm + 128],
                            lhsT=stg[:, mm, :], rhs=ident,
                            is_transpose=True,
                            start=(mm == 0), stop=(mm == 3),
                        )
                    # o2[64*par + j2, 128u + 32a + r'] =
                    #     trp[j2, 16r' + 2u + par]
                    pv_v = trp[0:64, :].rearrange(
                        "j (rr uu two) -> j two uu rr", two=2, uu=8)
                    o2_v = o2s[g][i].rearrange(
                        "q (u rr) -> q u rr", u=8)[:, :, 32 * a:32 * a + 32]
                    for par in range(2):
                        nc.vector.tensor_copy(
                            o2_v[64 * par:64 * par + 64], pv_v[:, par])
                    if i == 1:
                        pvs_by.pop((g, a))

                pending = None
                inter = []
                sched = []
                tail_parts = None
                rot = [0]

                def next_tag():
                    rot[0] += 1
                    return "stA" if rot[0] % 2 == 0 else "stB"

                units = [(g, a, gg, i) for g in range(NG) for a in range(4)
                         for gg in range(2 * a + 2) for i in range(2)]
                for ui, (g, a, gg, i) in enumerate(units):
                    if inter and sched and ui >= sched[0]:
                        sched.pop(0)
                        inter.pop(0)(next_tag())
                    if gg == 0 and i == 0:
                        pvs_by[(g, a)] = [
                            mmps.tile([128, 4, 65], F32, tag="mm",
                                      name=f"pv{g}_{a}_{ii}")
                            for ii in range(2)]
                    diag = gg >= 2 * a
                    d = gg - 2 * a
                    sts = stps.tile([128, 1024], F32, tag=next_tag(),
                                    name=f"st{g}_{a}_{gg}_{i}")
                    for q2 in range(2):
                        bb = 2 * gg + q2
                        # diagonal blocks m=1,2 only need cols >= 128m (m=3
                        # would drop the free dim under 256 for no gain)
                        m = 2 * d + q2 if diag else 0
                        off = 128 * m if m in (1, 2) else 0
                        nc.tensor.matmul(
                            sts[:, q2 * 512 + off:(q2 + 1) * 512],
                            lhsT=kht2[g][64 * i:64 * i + 64,
                                         bb * 128:(bb + 1) * 128],
                            rhs=qht2[g][64 * i:64 * i + 64,
                                        a * 512 + off:(a + 1) * 512],
                            start=True, stop=True,
                        )
                    pt = ptp.tile([128, 1024], BF16, tag="pt",
                                  name=f"pt_{g}_{a}_{gg}_{i}")
                    if not diag:
                        nc.scalar.activation(pt, sts, EXP)
                    elif d == 0:
                        # one full-width exp (cols [512:640) are garbage the
                        # restricted PV never reads); strips masked after
                        nc.scalar.activation(pt, sts, EXP)
                        nc.vector.tensor_mul(
                            pt[:, 0:128], pt[:, 0:128], tri01)
                        nc.vector.tensor_mul(
                            pt[:, 640:768], pt[:, 640:768], tri01)
                    else:
                        # exp only the valid columns; zero the boundary
                        # strip's upper triangle with the 0/1 mask
                        for q2 in range(2):
                            m = 2 * d + q2
                            lo = q2 * 512 + 128 * m
                            hi = (q2 + 1) * 512
                            nc.scalar.activation(
                                pt[:, lo:hi], sts[:, lo:hi], EXP)
                            nc.vector.tensor_mul(
                                pt[:, lo:lo + 128],
                                pt[:, lo:lo + 128], tri01)
                    if pending is not None:
                        emit_pv(*pending)
                        pg, pa, pgg, pi, _ = pending
                        if pgg == 2 * pa + 1:
                            # this pair's PV chain just completed
                            emit_norm_one(pg, pa, pi)
                            if pg == 0 and pa == 3 and pi == 1:
                                inter = emit_y_parts(0, o2s[0], wo_sb, ypool)
                                sched = [ui + 24, ui + 38]
                            if pg == 1 and pa == 3 and pi == 0:
                                # overlap g1's first output-projection half
                                # with the final unit's PV + normalization
                                tail_parts = emit_y_parts(1, o2s[1], wo_sb,
                                                          ypool)
                                tail_parts[0]()
                    pending = (g, a, gg, i, pt)
                emit_pv(*pending)
                emit_norm_one(pending[0], pending[1], pending[3])
                for ch in inter:  # safety: flush any unfired injections
                    ch(next_tag())
                tail_parts[1]()
                return o2s

            def emit_y_parts(g, o2, wo_sb, ypool):
                """Returns closures [part_jh0, part_jh1]; each emits half of
                the output projection so it can interleave with the next
                group's attention stream."""
                ysbs = [ypool.tile([128, 1024], BF16, tag="ysb",
                                   name=f"ysb{g}_{i}") for i in range(2)]

                def part(i, tag=None):
                    for jh in range(2):
                        yps = mmps.tile([128, 512], F32, tag="mm",
                                        name=f"ypsum_{i}_{jh}")
                        for u in range(8):
                            nc.tensor.matmul(
                                yps,
                                lhsT=o2[i][:, u * 128:(u + 1) * 128],
                                rhs=wo_sb[:, u * 1024 + jh * 512:
                                          u * 1024 + (jh + 1) * 512],
                                start=(u == 0), stop=(u == 7),
                            )
                        nc.vector.tensor_add(
                            ysbs[i][:, jh * 512:(jh + 1) * 512], yps,
                            bias_sb[:, jh * 512:(jh + 1) * 512])
                        nc.sync.dma_start(
                            out=out[2 * g + i][:, jh * 512:(jh + 1) * 512],
                            in_=ysbs[i][:, jh * 512:(jh + 1) * 512])

                return [lambda tag=None: part(0), lambda tag=None: part(1)]

            with (
                tc.tile_pool(name="xtp", bufs=1) as xtp,
                tc.tile_pool(name="wp", bufs=1) as wp,
                tc.tile_pool(name="pwork", bufs=1) as pwork,
                tc.tile_pool(name="p2", bufs=1) as p2,
                tc.tile_pool(name="ptp", bufs=6) as ptp,
                tc.tile_pool(name="rp", bufs=2) as rp,
                tc.tile_pool(name="o2p", bufs=2) as o2p,
                tc.tile_pool(name="yp", bufs=2) as ypool,
            ):
                xb_sb = xtp.tile([128, 8, PPC, 128], BF16, tag="xb",
                                 name="xbsb")
                xv = xtb.rearrange("p (kc pr r) -> p kc pr r", kc=8, pr=PPC)
                for kc in range(8):
                    nc.scalar.dma_start(out=xb_sb[:, kc], in_=xv[:, kc])

                w_sbs = []
                for wi, (wparam, dt_) in enumerate(
                        ((wv, BF16), (wq, BF16), (wk, BF16))):
                    w_sb = wp.tile([128, 8, 1024], dt_, tag=f"w{wi}",
                                   name=f"w{wi}")
                    for kc in range(8):
                        nc.sync.dma_start(
                            out=w_sb[:, kc, :],
                            in_=wparam.rearrange(
                                "(c p) j -> p c j", p=128)[:, kc, :])
                    w_sbs.append(w_sb)
                wo_sb = p2.tile([128, 8 * 1024], BF16, tag="wo")
                nc.sync.dma_start(out=wo_sb, in_=wo[:])

                # group 0 runs as early as possible; group 1's projections
                # and pairs 2/3's V path are injected into attention(g0)'s
                # ACT-bound stream at psum-quiet unit indices
                pre_rot = [0]

                def pre_tag():
                    pre_rot[0] += 1
                    return "stA" if pre_rot[0] % 2 == 0 else "stB"

                q0 = qk_chunks(w_sbs[1], xb_sb, qht2, 0, "q")
                q1 = qk_chunks(w_sbs[1], xb_sb, qht2, 1, "q")
                k0 = qk_chunks(w_sbs[2], xb_sb, kht2, 0, "k")
                k1 = qk_chunks(w_sbs[2], xb_sb, kht2, 1, "k")
                for ch in v_chunks(w_sbs[0], xb_sb, pwork, [0, 1]):
                    ch()
                for ch in (q0[0], q1[0], q0[1], q1[1]):
                    ch(pre_tag())
                for ch in (k0[0], k1[0], k0[1], k1[1]):
                    ch(pre_tag())
                for ch in v_chunks(w_sbs[0], xb_sb, pwork, [2, 3]):
                    ch()
                nc.scalar.dma_start(out=bias_sb, in_=bias[:])

                emit_attention_fused(ptp, rp, o2p, ypool, wo_sb)

    nc.finalize()
    return nc


def _host_prep(input_seq_embs, W_Q, W_K, W_V, W_O, b_O):
    X = np.asarray(input_seq_embs, dtype=np.float32)
    WQ = np.asarray(W_Q, dtype=np.float32)
    WK = np.asarray(W_K, dtype=np.float32)
    WV = np.asarray(W_V, dtype=np.float32)
    WO = np.asarray(W_O, dtype=np.float32)
    bO = np.asarray(b_O, dtype=np.float32)

    import ml_dtypes
    bf16 = ml_dtypes.bfloat16

    wq_arr = np.ascontiguousarray(WQ.T).astype(bf16)
    wk_arr = np.ascontiguousarray(WK.T).astype(bf16)
    wv_arr = np.ascontiguousarray(WV.T).astype(bf16)
    # wo[64*parity + j2, 1024*u + jo] = W_O.T[64*(2u+parity) + j2, jo]
    wo_arr = np.ascontiguousarray(
        WO.T.reshape(8, 2, 64, 1024).transpose(1, 2, 0, 3).reshape(
            128, 8192)).astype(bf16)
    bias_arr = np.ascontiguousarray(
        np.broadcast_to(bO, (128, 1024)).astype(np.float32))

    in_maps = []
    for c in range(NCORES):
        # xt[p_, kc, pair, r] = X_pair[r, 128*kc + p_]
        xts = np.empty((128, 8, PPC, 128), dtype=np.float32)
        for p in range(PPC):
            g = PPC * c + p
            bb, hh = g // H, g % H
            Xs = X[bb, 128 * hh:128 * (hh + 1), :]      # (128 r, 1024 cin)
            xts[:, :, p, :] = Xs.T.reshape(8, 128, 128).transpose(1, 0, 2)
        xt_arr = np.ascontiguousarray(xts.reshape(128, 8 * PPC * 128))
        in_maps.append({
            "xtb": xt_arr.astype(bf16),
            "wq": wq_arr, "wk": wk_arr, "wv": wv_arr, "wo": wo_arr,
            "bias": bias_arr,
        })
    return in_maps


_CACHED_NC = None


def get_nc():
    global _CACHED_NC
    if _CACHED_NC is None:
        _CACHED_NC = build_nc()
    return _CACHED_NC


def kernel(**inputs) -> np.ndarray:
    nc = get_nc()
    in_maps = _host_prep(**inputs)
    res = run_bass_kernel_spmd(nc, in_maps, list(range(NCORES)))
    out = np.empty((B, L, D), dtype=np.float32)
    for c in range(NCORES):
        y = np.asarray(res.results[c]["out"],
                       dtype=np.float32)  # (4, 128, 1024)
        for p in range(PPC):
            g = PPC * c + p
            bb, hh = g // H, g % H
            out[bb, 128 * hh:128 * (hh + 1), :] = y[p]
    return out

